# revision 2
# baseline (speedup 1.0000x reference)
"""Trainium2 Bass kernel for the grouped linear ensemble (moe_routing).

Problem: x [262144, 256] f32, Ws [64, 4, 256, 256], bs [64, 4, 256].
Model m applies its 4-layer stack (h = h @ W_l + b_l) to its contiguous
4096-row slice of x.

Sharding: expert parallel - core c owns models 8c..8c+7 and their rows.
No cross-device communication.

Per-core kernel design (v3):
- The 4-layer chain is affine; the host composes it into a single layer
  per model (Wc = W1 W2 W3 W4, bc folded likewise, in float64), so the
  device does one 256x256 GEMM per model slice.
- All device traffic is fp16 (tolerance 2e-2; fp16 keeps rel err ~4e-4).
  x is pre-transposed and pre-cast on the host to feature-major tiles
  with a chunk-local column permutation t' = j*128 + p <-> original row
  p*32 + j, which makes every store line contiguous in DRAM.
- One 4096-row chunk per model streams through: one 2 MB sync-ring
  (HWDGE) load of x^T [128, 2, 4096]; per pair of 128-row j-blocks the
  four matmuls accumulate y = x @ Wc into a [128, 512] f32 PSUM bank;
  DVE drains each bank with a single tensor_copy (f32 -> fp16).
- The bias is NOT added on device: y = x@Wc is stored biasless in fp16
  and the host adds the f32 bias during the final upcast. This removes
  the bias DMA and makes drains single-operand copies.
- Stores go on the scalar (ACT) HWDGE ring - disjoint from the load
  ring - as two 1 MB transfers per chunk (16 KB contiguous per
  partition line), issued as soon as each half-chunk is drained.
- Keep buffering shallow (h/o pools 3, PSUM pool 6): measured on this
  platform, deeper pools and finer store splits reproducibly fall into
  a ~2x slower regime.
"""
from contextlib import ExitStack

import numpy as np

import concourse.tile as tile
import concourse.mybir as mybir
from concourse import bacc
from concourse.bass_utils import run_bass_kernel_spmd

N_CORES = 8
N_MODELS = 64
N_LAYERS = 4
F = 256
ROWS_PER_MODEL = 4096
M_PER_CORE = N_MODELS // N_CORES             # 8 models per core
ROWS_PER_CORE = M_PER_CORE * ROWS_PER_MODEL  # 32768
CHUNK = 4096                                 # rows per pipeline step
JG = CHUNK // 128                            # 32 j-blocks per chunk
JP = JG // 2                                 # 16 psum-pair tiles per chunk
CHUNKS_PER_MODEL = ROWS_PER_MODEL // CHUNK   # 1
STORE_HALVES = 2

F32 = mybir.dt.float32
FP16 = mybir.dt.float16


def emit_core_kernel(tc, xT_d, wc_d, y_d, reps=1):
    nc = tc.nc
    ctx = ExitStack()
    wpool = ctx.enter_context(tc.tile_pool(name="w", bufs=2))
    hpool = ctx.enter_context(tc.tile_pool(name="h", bufs=3))
    opool = ctx.enter_context(tc.tile_pool(name="o", bufs=3))
    psL = ctx.enter_context(tc.tile_pool(name="psL", bufs=6, space="PSUM"))

    def body():
        for m in range(M_PER_CORE):
            wc = []
            for fb in range(2):
                wr = wpool.tile([128, F], FP16, tag=f"wr_{fb}")
                nc.sync.dma_start(wr[:], wc_d[m, fb * 128:(fb + 1) * 128, :])
                wc.append(wr)

            for c in range(CHUNKS_PER_MODEL):
                r0 = (m * CHUNKS_PER_MODEL + c) * CHUNK
                h = hpool.tile([128, 2, CHUNK], FP16, tag="h")
                xv = xT_d[:, r0:r0 + CHUNK].rearrange("(fb p) t -> p fb t",
                                                      fb=2)
                nc.sync.dma_start(h[:], xv)
                on = opool.tile([128, JG, F], FP16, tag="on")
                ov = on[:].rearrange("p (jp two) f -> p jp (two f)", two=2)
                jsz = JG // STORE_HALVES
                for jp in range(JP):
                    p4 = psL.tile([128, 2 * F], F32, tag="p4")
                    for half in range(2):
                        j = 2 * jp + half
                        for fb in range(2):
                            nc.tensor.matmul(
                                p4[:, half * F:(half + 1) * F],
                                h[:, fb, j * 128:(j + 1) * 128],
                                wc[fb][:],
                                start=(fb == 0),
                                stop=(fb == 1),
                            )
                    nc.vector.tensor_copy(ov[:, jp], p4[:])
                    # store each half-chunk as soon as it is drained
                    jdone = (jp + 1) * 2
                    if jdone % jsz == 0:
                        j0 = jdone - jsz
                        yv = y_d[r0:r0 + CHUNK, :].rearrange(
                            "(p j) f -> p j f", j=JG)
                        nc.scalar.dma_start(yv[:, j0:jdone], on[:, j0:jdone])

    if reps == 1:
        body()
    else:
        with tc.For_i(0, reps, 1):
            body()
    ctx.close()


def build_nc(reps=1, num_devices=N_CORES):
    nc = bacc.Bacc("TRN2", target_bir_lowering=False, debug=False,
                   num_devices=num_devices)
    xT_d = nc.dram_tensor("xT", [F, ROWS_PER_CORE], FP16,
                          kind="ExternalInput").ap()
    wc_d = nc.dram_tensor("Wc", [M_PER_CORE, F, F], FP16,
                          kind="ExternalInput").ap()
    y_d = nc.dram_tensor("y", [ROWS_PER_CORE, F], FP16,
                         kind="ExternalOutput").ap()
    with tile.TileContext(nc) as tc:
        emit_core_kernel(tc, xT_d, wc_d, y_d, reps=reps)
    nc.compile()
    return nc


_NC = None


def _get_nc():
    global _NC
    if _NC is None:
        _NC = build_nc()
    return _NC


def _compose_affine(Ws, bs):
    """Fold the 4-layer affine chain into one layer per model (float64)."""
    W = np.asarray(Ws, dtype=np.float64)
    b = np.asarray(bs, dtype=np.float64)
    Wc = W[:, 0]
    bc = b[:, 0]
    for l in range(1, N_LAYERS):
        Wc = np.matmul(Wc, W[:, l])
        bc = np.matmul(bc[:, None, :], W[:, l])[:, 0] + b[:, l]
    return Wc, bc


def make_in_maps(x, Ws, bs):
    Wc, _ = _compose_affine(Ws, bs)
    Wch = Wc.astype(np.float16)
    xh = np.asarray(x, dtype=np.float16)
    in_maps = []
    for c in range(N_CORES):
        m0 = c * M_PER_CORE
        r0 = m0 * ROWS_PER_CORE // M_PER_CORE * M_PER_CORE  # = c*ROWS_PER_CORE
        r0 = c * ROWS_PER_CORE
        # feature-major x with chunk-local column permutation t' = j*128 + p
        # holding original row p*JG + j: j-block j's matmul emits rows
        # {p*JG + j}, so partition p's store run is contiguous in DRAM.
        xc = xh[r0:r0 + ROWS_PER_CORE].reshape(-1, 128, JG, F)
        xTp = np.ascontiguousarray(
            xc.transpose(3, 0, 2, 1).reshape(F, ROWS_PER_CORE))
        in_maps.append({
            "xT": xTp,
            "Wc": np.ascontiguousarray(Wch[m0:m0 + M_PER_CORE]),
        })
    return in_maps


def kernel(x, Ws, bs, slice_bounds=None, **_):
    x = np.asarray(x, dtype=np.float32)
    Ws = np.asarray(Ws, dtype=np.float32)
    bs = np.asarray(bs, dtype=np.float32)
    nc = _get_nc()
    res = run_bass_kernel_spmd(nc, make_in_maps(x, Ws, bs),
                               core_ids=list(range(N_CORES)))
    _, bc = _compose_affine(Ws, bs)
    bc32 = bc.astype(np.float32)  # [N_MODELS, F], added on host (biasless y
    # on device; fp16 quantization applies to x@Wc only)
    out = np.empty((N_MODELS * ROWS_PER_MODEL, F), dtype=np.float32)
    for c in range(N_CORES):
        yc = res.results[c]["y"].astype(np.float32).reshape(
            M_PER_CORE, ROWS_PER_MODEL, F)
        yc += bc32[c * M_PER_CORE:(c + 1) * M_PER_CORE, None, :]
        out[c * ROWS_PER_CORE:(c + 1) * ROWS_PER_CORE] = yc.reshape(
            ROWS_PER_CORE, F)
    return out
